# revision 1
# baseline (speedup 1.0000x reference)
"""Trainium2 Bass kernel for the BDH-style sparse-attention network.

Reference computation (per batch b, all fp32):
  v = LN(wte[idx])                                   [T, D]
  repeat L times:
    x   = relu(v @ Dx)                               [T, N]   (Dx: [D, N] = decoder_x heads concat)
    a   = causal_linear_attention(v) (RoPE, no softmax, tril mask)
    y   = relu(LN(a) @ Dy) * x                       [T, N]
    v   = v + LN(y @ E)                              [T, D]   (E: [N, D] = encoder)
  logits = v @ readout                               [T, VOCAB]

Sharding over 8 NeuronCores: core c -> batch b = c//2, neuron half h = c%2.
Each core holds half the neuron dim (N/2 columns of Dx/Dy, N/2 rows of E) and
computes the full attention for its batch; the partial `y @ E` update is
summed with an AllReduce over core pairs [[0,1],[2,3],[4,5],[6,7]].

On-device layout: token-major tiles v [128t, D] plus a transposed copy
vT [128d, T] maintained via PE transposes, so every matmul has its
contraction dim on partitions without extra data movement.
"""

import numpy as np

import concourse.bass as bass
import concourse.bacc as bacc
import concourse.mybir as mybir
import concourse.tile as tile
from concourse.bass_utils import run_bass_kernel_spmd

FP = mybir.dt.float32
AX = mybir.AxisListType
ALU = mybir.AluOpType
ACTF = mybir.ActivationFunctionType
EPS = 1e-5


def default_cfg():
    return dict(
        T=2048, D=256, N=8192, H=4, VOCAB=256, L=6, B=4,
        TCHUNK=512,          # tokens per chunk == attention query block
        mm_dt="f32r",        # "f32r" | "f32" : dtype view fed to the PE
        w_dt="mm",           # "bf16" | "mm" : matmul dtype for the MLP path
        n_cores=8,
        reps=1,              # layer-stack repeats (for wall-clock timing deltas)
    )


def build_program(cfg):
    """Builds and compiles the per-core SPMD bass program."""
    T, D, VOCAB, L = cfg["T"], cfg["D"], cfg["VOCAB"], cfg["L"]
    NH = cfg["N"] // 2
    TC = cfg["TCHUNK"]
    TT = T // 128
    DT = D // 128
    VT = VOCAB // 128
    n_cores = cfg["n_cores"]
    assert D == 256 and TC % 128 == 0 and T % TC == 0 and T % 512 == 0

    MDT = mybir.dt.float32r if cfg["mm_dt"] == "f32r" else FP
    WDT = mybir.dt.bfloat16 if cfg.get("w_dt") == "bf16" else MDT

    nc = bacc.Bacc("TRN2", target_bir_lowering=False, debug=False,
                   num_devices=n_cores)

    idxf_d = nc.dram_tensor("idxf", [1, T], FP, kind="ExternalInput")
    wte_d = nc.dram_tensor("wte", [VT, 128, D], FP, kind="ExternalInput")
    dxh_d = nc.dram_tensor("dxh", [DT, 128, NH], WDT, kind="ExternalInput")
    dyh_d = nc.dram_tensor("dyh", [DT, 128, NH], WDT, kind="ExternalInput")
    eh_d = nc.dram_tensor("eh", [NH // 128, 128, D], WDT, kind="ExternalInput")
    ro_d = nc.dram_tensor("ro", [DT, 128, VOCAB], WDT, kind="ExternalInput")
    cosT_d = nc.dram_tensor("cosT", [DT, 128, T], WDT, kind="ExternalInput")
    sinT_d = nc.dram_tensor("sinT", [DT, 128, T], WDT, kind="ExternalInput")
    ident_d = nc.dram_tensor("ident", [128, 128], MDT, kind="ExternalInput")
    logits_d = nc.dram_tensor("logits", [TT, 128, VOCAB], FP,
                              kind="ExternalOutput")

    groups = [[2 * i, 2 * i + 1] for i in range(n_cores // 2)]

    with tile.TileContext(nc) as tc:
        with (
            tc.tile_pool(name="pers", bufs=1) as pers,
            tc.tile_pool(name="wk", bufs=3) as wk,
            tc.tile_pool(name="lat", bufs=2) as latp,
            tc.tile_pool(name="sm", bufs=4) as sm,
            tc.tile_pool(name="col", bufs=6) as col,
            tc.tile_pool(name="ps", bufs=4, space="PSUM") as ps,
            tc.tile_pool(name="acc", bufs=2, space="PSUM") as acc,
            tc.tile_pool(name="dram", bufs=1, space="DRAM") as dram,
        ):
            env = dict(nc=nc, cfg=cfg, MDT=MDT, WDT=WDT, wk=wk, sm=sm, col=col,
                       ps=ps, acc=acc, latp=latp, groups=groups, eh_d=eh_d,
                       cosT_d=cosT_d, sinT_d=sinT_d)

            # ---------- persistent SBUF ----------
            ident = pers.tile([128, 128], MDT, tag="ident", name="ident")
            nc.sync.dma_start(ident[:], ident_d[:])
            env["ident"] = ident

            eps_col = pers.tile([128, 1], FP, tag="eps", name="eps_col")
            nc.vector.memset(eps_col[:], EPS)
            env["eps_col"] = eps_col

            idxf = pers.tile([1, T], FP, tag="idxf", name="idxf")
            nc.sync.dma_start(idxf[:], idxf_d[:])
            wte = []
            for i in range(VT):
                w = pers.tile([128, D], FP, tag=f"wte{i}", name=f"wte{i}")
                nc.sync.dma_start(w[:], wte_d[i])
                wte.append(w)

            env["dxh"] = dxh = []
            env["dyh"] = dyh = []
            for i in range(DT):
                dx = pers.tile([128, NH], WDT, tag=f"dxh{i}", name=f"dxh{i}")
                dy = pers.tile([128, NH], WDT, tag=f"dyh{i}", name=f"dyh{i}")
                nc.sync.dma_start(dx[:], dxh_d[i])
                nc.sync.dma_start(dy[:], dyh_d[i])
                dxh.append(dx)
                dyh.append(dy)

            ro = []
            for i in range(DT):
                r = pers.tile([128, VOCAB], WDT, tag=f"ro{i}", name=f"ro{i}")
                nc.sync.dma_start(r[:], ro_d[i])
                ro.append(r)

            if cfg.get("w_dt") == "bf16":
                env["ehs"] = ehs = []
                for m in range(NH // 128):
                    e = pers.tile([128, D], WDT, tag=f"ehs{m}", name=f"ehs{m}")
                    nc.sync.dma_start(e[:], eh_d[m])
                    ehs.append(e)
            else:
                env["ehs"] = None

            env["v_sb"] = v_sb = [
                pers.tile([128, D], MDT, tag=f"v{t}", name=f"v{t}")
                for t in range(TT)]
            env["vT"] = vT = [
                pers.tile([128, T], WDT, tag=f"vT{i}", name=f"vT{i}")
                for i in range(DT)]
            env["qT"] = [
                pers.tile([128, T], MDT, tag=f"qT{i}", name=f"qT{i}")
                for i in range(DT)]

            # ---------- embedding ----------
            lnwte = []
            for i in range(VT):
                lw = pers.tile([128, D], MDT, tag=f"lnwte{i}", name=f"lnwte{i}")
                _ln_rows(env, lw, wte[i], D)
                lnwte.append(lw)

            ones1 = pers.tile([1, 128], FP, tag="ones1", name="ones1")
            nc.vector.memset(ones1[:], 1.0)
            iotav = []
            for i in range(VT):
                iv = pers.tile([128, 1], FP, tag=f"iotav{i}", name=f"iotav{i}")
                nc.gpsimd.iota(iv[:], pattern=[[0, 1]], base=i * 128,
                               channel_multiplier=1,
                               allow_small_or_imprecise_dtypes=True)
                iotav.append(iv)

            # onehotT[v, t] = (idx[t] == v), built and consumed per 512-chunk
            with tc.tile_pool(name="emb", bufs=2) as embp:
                for c in range(T // 512):
                    cs = slice(c * 512, (c + 1) * 512)
                    pidx = ps.tile([128, 512], FP, tag="mm", name="pidx")
                    nc.tensor.matmul(pidx[:], ones1[:], idxf[:, cs],
                                     start=True, stop=True)
                    oh = []
                    for i in range(VT):
                        ohi = embp.tile([128, 512], MDT, tag="ohs", name="ohs")
                        nc.vector.tensor_scalar(ohi[:], pidx[:], iotav[i][:],
                                                None, op0=ALU.is_equal)
                        oh.append(ohi)
                    # v0 = LN(wte)[idx] for the 4 token tiles of this chunk
                    for tl in range(4):
                        t = c * 4 + tl
                        pv = ps.tile([128, D], FP, tag="mm", name="pv")
                        for i in range(VT):
                            nc.tensor.matmul(pv[:],
                                             (oh[i][:, tl * 128:(tl + 1) * 128]),
                                             (lnwte[i][:]),
                                             start=(i == 0), stop=(i == VT - 1))
                        nc.vector.tensor_copy(v_sb[t][:], pv[:])
                    for i in range(DT):
                        pvt = ps.tile([128, 512], FP, tag="mm", name="pvt")
                        for k in range(VT):
                            nc.tensor.matmul(
                                pvt[:],
                                (lnwte[k][:, i * 128:(i + 1) * 128]),
                                (oh[k][:]),
                                start=(k == 0), stop=(k == VT - 1))
                        nc.vector.tensor_copy(vT[i][:, cs], pvt[:])

            env["up_dram"] = dram.tile([T, D], FP, name="upd")
            env["upr_dram"] = dram.tile([T, D], FP, name="uprd")

            # ---------- layers ----------
            total_layers = cfg["reps"] * L
            for li in range(total_layers):
                env["_layers_left"] = total_layers - 1 - li
                _emit_layer(env)

            # ---------- readout ----------
            for t in range(TT):
                pl = ps.tile([128, VOCAB], FP, tag="mm", name="pl")
                for i in range(DT):
                    nc.tensor.matmul(pl[:],
                                     (vT[i][:, t * 128:(t + 1) * 128]),
                                     (ro[i][:]),
                                     start=(i == 0), stop=(i == DT - 1))
                lg = wk.tile([128, VOCAB], FP, tag="lg", name="lg")
                nc.vector.tensor_copy(lg[:], pl[:])
                nc.sync.dma_start(logits_d[t], lg[:])

    nc.compile()
    return nc


def _ln_rows(env, out_ap, in_ap, F, resid_ap=None):
    """LN over the free dim per partition row. If resid_ap: out = resid + ln(in)."""
    nc, sm, col = env["nc"], env["sm"], env["col"]
    st6 = col.tile([128, 6], FP, tag="bst", name="bst")
    nc.vector.bn_stats(st6[:], in_ap[:])
    st2 = col.tile([128, 2], FP, tag="bag", name="bag")
    nc.vector.bn_aggr(st2[:], st6[:])
    std = col.tile([128, 1], FP, tag="std", name="std")
    nc.scalar.activation(std[:], st2[:, 1:2], ACTF.Sqrt, bias=env["eps_col"][:])
    rstd = col.tile([128, 1], FP, tag="rstd", name="rstd")
    nc.vector.reciprocal(rstd[:], std[:])
    if resid_ap is None:
        nc.vector.tensor_scalar(out_ap[:], in_ap[:], st2[:, 0:1], rstd[:],
                                op0=ALU.subtract, op1=ALU.mult)
    else:
        tmp = sm.tile([128, F], FP, tag="lntmp", name="lntmp")
        nc.vector.tensor_scalar(tmp[:], in_ap[:], st2[:, 0:1], rstd[:],
                                op0=ALU.subtract, op1=ALU.mult)
        nc.vector.tensor_add(out_ap[:], resid_ap[:], tmp[:])


def _emit_rope(env, g):
    nc, WDT = env["nc"], env["WDT"]
    cfg = env["cfg"]
    TC = cfg["TCHUNK"]
    DT = cfg["D"] // 128
    wk, vT, qT = env["wk"], env["vT"], env["qT"]
    cosT_d, sinT_d = env["cosT_d"], env["sinT_d"]
    cs = slice(g * TC, (g + 1) * TC)
    for i in range(DT):
        o = 1 - i
        ctab = wk.tile([128, TC], WDT, tag="ctab", bufs=2, name="ctab")
        nc.sync.dma_start(ctab[:], cosT_d[i, :, cs])
        stab = wk.tile([128, TC], WDT, tag="stab", bufs=2, name="stab")
        nc.sync.dma_start(stab[:], sinT_d[i, :, cs])
        t1 = wk.tile([128, TC], FP, tag="rope", bufs=2, name="ropeA")
        nc.vector.tensor_mul(t1[:], vT[i][:, cs], ctab[:])
        t2 = wk.tile([128, TC], FP, tag="rope", bufs=2, name="ropeB")
        nc.vector.tensor_mul(t2[:], vT[o][:, cs], stab[:])
        if i == 0:
            nc.vector.tensor_sub(qT[i][:, cs], t1[:], t2[:])
        else:
            nc.vector.tensor_add(qT[i][:, cs], t1[:], t2[:])


def _emit_layer(env):
    nc, cfg, MDT = env["nc"], env["cfg"], env["MDT"]
    WDT = env["WDT"]
    T, D = cfg["T"], cfg["D"]
    NH = cfg["N"] // 2
    TC = cfg["TCHUNK"]
    G = T // TC
    TT = T // 128
    MC = NH // 128
    NSUB = TC // 128
    DT = D // 128
    wk, sm, col, ps, acc = env["wk"], env["sm"], env["col"], env["ps"], env["acc"]
    vT, qT, v_sb, latp = env["vT"], env["qT"], env["v_sb"], env["latp"]
    dxh, dyh = env["dxh"], env["dyh"]
    ident, eh_d = env["ident"], env["eh_d"]
    cosT_d, sinT_d = env["cosT_d"], env["sinT_d"]
    up_dram, upr_dram = env["up_dram"], env["upr_dram"]
    XPRE = 3                              # x m-chunks emitted ahead of y

    def emit_x(m, t0, xrs):
        x_ps = ps.tile([128, TC], FP, tag="mm", name="x_ps")
        for i in range(DT):
            nc.tensor.matmul(x_ps[:],
                             dxh[i][:, m * 128:(m + 1) * 128],
                             vT[i][:, t0:t0 + TC],
                             start=(i == 0), stop=(i == DT - 1))
        xr = wk.tile([128, TC], FP, tag="xr", bufs=4, name="xr")
        nc.scalar.activation(xr[:], x_ps[:], ACTF.Relu)
        xrs[m] = xr

    def emit_av(sb, e_sb, aT_ps, nkb):
        # aT[d, t] += v[s, d]^T e[s, t]  (free dim TC keeps f32r at full rate)
        for i in range(DT):
            nc.tensor.matmul(aT_ps[:, i, :],
                             v_sb[sb][:, i * 128:(i + 1) * 128],
                             e_sb[:],
                             start=(sb == 0), stop=(sb == nkb - 1),
                             skip_group_check=True)

    for g in range(G):
        t0 = g * TC
        nkb = (g + 1) * NSUB              # causal key-block count
        # ---- RoPE for this chunk (chunk 0 may have been emitted by prev layer)
        if not (g == 0 and env.pop("_rope0_done", False)):
            _emit_rope(env, g)
        # ---- attention: energyT[s, t] blocks, AV lags two blocks (PE pipelining)
        aT_ps = acc.tile([128, DT, TC], FP, tag="acc", name="aT_ps")
        pend = []
        for sb in range(nkb):
            e_ps = ps.tile([128, TC], FP, tag="mm", name="e_ps")
            for i in range(DT):
                nc.tensor.matmul(e_ps[:],
                                 qT[i][:, sb * 128:(sb + 1) * 128],
                                 qT[i][:, t0:t0 + TC],
                                 start=(i == 0), stop=(i == DT - 1))
            e_sb = wk.tile([128, TC], MDT, tag="esb", bufs=3, name="e_sb")
            nc.vector.tensor_copy(e_sb[:], e_ps[:])
            diag_j = sb - (nkb - NSUB)
            if diag_j >= 0:
                # causal mask: keep where t - s - 128*j >= 0
                nc.gpsimd.affine_select(e_sb[:], e_sb[:], pattern=[[1, TC]],
                                        compare_op=ALU.is_ge, fill=0.0,
                                        base=-128 * diag_j,
                                        channel_multiplier=-1)
            pend.append((sb, e_sb))
            if len(pend) > 2:
                s0, e0 = pend.pop(0)
                emit_av(s0, e0, aT_ps, nkb)
        xrs = {}
        emit_x(0, t0, xrs)
        for s0, e0 in pend:
            emit_av(s0, e0, aT_ps, nkb)
        pend = []
        emit_x(1, t0, xrs)
        # ---- aT -> a (PE transposes), LN(a), la -> laT slices ----
        aT_sb = [wk.tile([128, TC], MDT, tag=f"aTs{i}", bufs=2, name="aT_sb")
                 for i in range(DT)]
        for i in range(DT):
            nc.scalar.copy(aT_sb[i][:], aT_ps[:, i, :])
        emit_x(2, t0, xrs)
        laT = [latp.tile([128, TC], WDT, tag=f"laTs{i}", name=f"laT{i}")
               for i in range(DT)]
        for tsub in range(NSUB):
            a_ti = ps.tile([128, D], MDT, tag="mm", name="a_ti")
            for i in range(DT):
                nc.tensor.matmul(a_ti[:, i * 128:(i + 1) * 128],
                                 aT_sb[i][:, tsub * 128:(tsub + 1) * 128],
                                 ident[:], is_transpose=True,
                                 start=(i == 0), stop=(i == DT - 1),
                                 skip_group_check=True)
            la = sm.tile([128, D], MDT, tag="la", name="la")
            _ln_rows(env, la, a_ti, D)
            for i in range(DT):
                ptr = ps.tile([128, 128], MDT, tag="mm", name="ptr")
                nc.tensor.matmul(ptr[:], la[:, i * 128:(i + 1) * 128], ident[:],
                                 is_transpose=True, start=True, stop=True)
                nc.vector.tensor_copy(
                    laT[i][:, tsub * 128:(tsub + 1) * 128], ptr[:])
        # ---- MLP: y = relu(Dy^T laT) * x, updateT += Eh^T yel, x pipelined ahead
        upT_ps = acc.tile([128, DT, TC], FP, tag="acc", name="upT_ps")

        def emit_up(m, yel, ehm):
            for i in range(DT):
                nc.tensor.matmul(upT_ps[:, i, :],
                                 ehm[:, i * 128:(i + 1) * 128],
                                 yel[:],
                                 start=(m == 0), stop=(m == MC - 1),
                                 skip_group_check=True)

        pup = None
        for m in range(MC):
            if env["ehs"] is not None:
                ehm = env["ehs"][m]
            else:
                ehm = wk.tile([128, D], WDT, tag="ehst", bufs=4, name="ehm")
                nc.sync.dma_start(ehm[:], eh_d[m])
            if m + XPRE < MC:
                emit_x(m + XPRE, t0, xrs)
            y_ps = ps.tile([128, TC], FP, tag="mm", name="y_ps")
            for i in range(DT):
                nc.tensor.matmul(y_ps[:],
                                 dyh[i][:, m * 128:(m + 1) * 128],
                                 laT[i][:],
                                 start=(i == 0), stop=(i == DT - 1))
            yr = wk.tile([128, TC], FP, tag="yr", bufs=2, name="yr")
            nc.scalar.activation(yr[:], y_ps[:], ACTF.Relu)
            yel = wk.tile([128, TC], WDT, tag="yel", bufs=2, name="yel")
            nc.vector.tensor_mul(yel[:], yr[:], xrs.pop(m)[:])
            if pup is not None:
                emit_up(*pup)
            pup = (m, yel, ehm)
        emit_up(*pup)
        # ---- upT -> up (PE transposes), evacuate chunk to DRAM ----
        upT_sb = [wk.tile([128, TC], MDT, tag=f"uTs{i}", bufs=1, name="upT_sb")
                  for i in range(DT)]
        for i in range(DT):
            nc.scalar.copy(upT_sb[i][:], upT_ps[:, i, :])
        for tsub in range(NSUB):
            u_ti = ps.tile([128, D], MDT, tag="mm", name="u_ti")
            for i in range(DT):
                nc.tensor.matmul(u_ti[:, i * 128:(i + 1) * 128],
                                 upT_sb[i][:, tsub * 128:(tsub + 1) * 128],
                                 ident[:], is_transpose=True,
                                 start=(i == 0), stop=(i == DT - 1),
                                 skip_group_check=True)
            u_sb = wk.tile([128, D], FP, tag="usb", bufs=2, name="u_sb")
            nc.vector.tensor_copy(u_sb[:], u_ti[:])
            r0 = t0 + tsub * 128
            nc.sync.dma_start(up_dram[r0:r0 + 128], u_sb[:])
        # ---- AllReduce each completed half so it overlaps later chunks
        if g % 2 == 1 or g == G - 1:
            hs = slice(env.get("_ar_start", 0), (g + 1) * TC)
            env["_ar_start"] = (g + 1) * TC if g < G - 1 else 0
            if cfg.get("no_cc"):
                nc.sync.dma_start(upr_dram[hs], up_dram[hs])
            else:
                nc.gpsimd.collective_compute(
                    "AllReduce", ALU.add, replica_groups=env["groups"],
                    ins=[up_dram[hs].opt()], outs=[upr_dram[hs].opt()])

    # ---- v += LN(update); refresh vT; next layer's chunk-0 rope between halves
    if G > 1:
        nfirst = TC // 128            # rope(0) needs only vT tiles of chunk 0
        _emit_vnew(env, 0, nfirst)
        if env.get("_layers_left", 0) > 0:
            _emit_rope(env, 0)
            env["_rope0_done"] = True
        _emit_vnew(env, nfirst, TT)
    else:
        _emit_vnew(env, 0, TT)


def _emit_vnew(env, t_lo, t_hi):
    nc, MDT = env["nc"], env["MDT"]
    D = env["cfg"]["D"]
    DT = D // 128
    sm, ps = env["sm"], env["ps"]
    v_sb, vT, ident = env["v_sb"], env["vT"], env["ident"]
    upr_dram = env["upr_dram"]
    for t in range(t_lo, t_hi):
        upr = sm.tile([128, D], FP, tag="upr", name="upr")
        nc.sync.dma_start(upr[:], upr_dram[t * 128:(t + 1) * 128])
        _ln_rows(env, v_sb[t], upr, D, resid_ap=v_sb[t])
        for i in range(DT):
            ptr = ps.tile([128, 128], MDT, tag="mm", name="ptr2")
            nc.tensor.matmul(ptr[:], v_sb[t][:, i * 128:(i + 1) * 128], ident[:],
                             is_transpose=True, start=True, stop=True)
            nc.vector.tensor_copy(vT[i][:, t * 128:(t + 1) * 128], ptr[:])


# ====================== host side ======================

_BUILD_CACHE = {}


def shard_inputs(cfg, idx, wte, encoder, decoder_x, decoder_y, readout):
    """Returns the list of per-core input dicts."""
    import ml_dtypes
    wnp = ml_dtypes.bfloat16 if cfg.get("w_dt") == "bf16" else np.float32
    T, D, VOCAB = cfg["T"], cfg["D"], cfg["VOCAB"]
    NH = cfg["N"] // 2
    DT = D // 128
    VT = VOCAB // 128
    heads_per_half = cfg["H"] // 2

    inv_freq = 1.0 / (10000.0 ** (np.arange(0, D, 2, dtype=np.float64) / D))
    tpos = np.arange(T, dtype=np.float64)
    freqs = np.outer(tpos, inv_freq)
    emb = np.concatenate([freqs, freqs], axis=-1)     # [T, D]
    cosT = np.ascontiguousarray(np.cos(emb).T.astype(wnp)).reshape(DT, 128, T)
    sinT = np.ascontiguousarray(np.sin(emb).T.astype(wnp)).reshape(DT, 128, T)
    ident = np.eye(128, dtype=np.float32)

    wte_s = np.ascontiguousarray(wte.astype(np.float32)).reshape(VT, 128, D)
    ro_s = np.ascontiguousarray(readout.astype(wnp)).reshape(DT, 128, VOCAB)

    in_maps = []
    for c in range(cfg["n_cores"]):
        b, h = c // 2, c % 2
        heads = range(h * heads_per_half, (h + 1) * heads_per_half)
        dxh = np.concatenate([decoder_x[hh] for hh in heads], axis=-1)  # [D, NH]
        dyh = np.concatenate([decoder_y[hh] for hh in heads], axis=-1)
        ehh = encoder[h * NH:(h + 1) * NH]                              # [NH, D]
        in_maps.append(dict(
            idxf=np.ascontiguousarray(idx[b].astype(np.float32)[None, :]),
            wte=wte_s,
            dxh=np.ascontiguousarray(dxh.astype(wnp)).reshape(DT, 128, NH),
            dyh=np.ascontiguousarray(dyh.astype(wnp)).reshape(DT, 128, NH),
            eh=np.ascontiguousarray(ehh.astype(wnp)).reshape(NH // 128, 128, D),
            ro=ro_s,
            cosT=cosT, sinT=sinT, ident=ident,
        ))
    return in_maps


def get_program(cfg):
    key = (cfg["T"], cfg["N"], cfg["L"], cfg["mm_dt"], cfg.get("w_dt"),
           cfg["reps"], cfg["n_cores"], cfg["TCHUNK"], cfg.get("no_cc"))
    if key not in _BUILD_CACHE:
        _BUILD_CACHE[key] = build_program(cfg)
    return _BUILD_CACHE[key]


def kernel(idx, wte, encoder, decoder_x, decoder_y, readout):
    cfg = default_cfg()
    nc = get_program(cfg)
    in_maps = shard_inputs(cfg, np.asarray(idx), np.asarray(wte),
                           np.asarray(encoder), np.asarray(decoder_x),
                           np.asarray(decoder_y), np.asarray(readout))
    res = run_bass_kernel_spmd(nc, in_maps, list(range(cfg["n_cores"])))
    B, T, VOCAB = cfg["B"], cfg["T"], cfg["VOCAB"]
    out = np.empty((B, T, VOCAB), np.float32)
    for b in range(B):
        out[b] = res.results[2 * b]["logits"].reshape(T, VOCAB)
    return out



# revision 11
# speedup vs baseline: 436.4515x; 436.4515x over previous
"""Trainium2 Bass kernel for the BDH-style sparse-attention network.

Reference computation (per batch b, all fp32):
  v = LN(wte[idx])                                   [T, D]
  repeat L times:
    x   = relu(v @ Dx)                               [T, N]   (Dx: [D, N] = decoder_x heads concat)
    a   = causal_linear_attention(v) (RoPE, no softmax, tril mask)
    y   = relu(LN(a) @ Dy) * x                       [T, N]
    v   = v + LN(y @ E)                              [T, D]   (E: [N, D] = encoder)
  logits = v @ readout                               [T, VOCAB]

Sharding over 8 NeuronCores: core c -> batch b = c//2, neuron half h = c%2.
Each core holds half the neuron dim (N/2 columns of Dx/Dy, N/2 rows of E) and
computes the full attention for its batch; the partial `y @ E` update is
summed with an AllReduce over core pairs [[0,1],[2,3],[4,5],[6,7]].

On-device layout: token-major tiles v [128t, D] plus a transposed copy
vT [128d, T] maintained via PE transposes, so every matmul has its
contraction dim on partitions without extra data movement.
"""

import numpy as np

import concourse.bass as bass
import concourse.bacc as bacc
import concourse.mybir as mybir
import concourse.tile as tile
from concourse.bass_utils import run_bass_kernel_spmd

FP = mybir.dt.float32
AX = mybir.AxisListType
ALU = mybir.AluOpType
ACTF = mybir.ActivationFunctionType
EPS = 1e-5


def default_cfg():
    return dict(
        T=2048, D=256, N=8192, H=4, VOCAB=256, L=6, B=4,
        TCHUNK=512,          # tokens per chunk == attention query block
        mm_dt="f32r",        # "f32r" | "f32" : dtype view fed to the PE
        w_dt="mm",           # "bf16" | "mm" : matmul dtype for the MLP path
        sched="pipe",        # "pipe" | "orig": collective-overlap pipeline
        n_cores=8,
        reps=1,              # layer-stack repeats (for wall-clock timing deltas)
    )


def build_program(cfg):
    """Builds and compiles the per-core SPMD bass program."""
    T, D, VOCAB, L = cfg["T"], cfg["D"], cfg["VOCAB"], cfg["L"]
    NH = cfg["N"] // 2
    TC = cfg["TCHUNK"]
    TT = T // 128
    DT = D // 128
    VT = VOCAB // 128
    n_cores = cfg["n_cores"]
    assert D == 256 and TC % 128 == 0 and T % TC == 0 and T % 512 == 0

    MDT = mybir.dt.float32r if cfg["mm_dt"] == "f32r" else FP
    WDT = mybir.dt.bfloat16 if cfg.get("w_dt") == "bf16" else MDT

    nc = bacc.Bacc("TRN2", target_bir_lowering=False, debug=False,
                   num_devices=n_cores)

    idxf_d = nc.dram_tensor("idxf", [1, T], FP, kind="ExternalInput")
    wte_d = nc.dram_tensor("wte", [VT, 128, D], FP, kind="ExternalInput")
    dxh_d = nc.dram_tensor("dxh", [DT, 128, NH], WDT, kind="ExternalInput")
    dyh_d = nc.dram_tensor("dyh", [DT, 128, NH], WDT, kind="ExternalInput")
    eh_d = nc.dram_tensor("eh", [NH // 128, 128, D], WDT, kind="ExternalInput")
    ro_d = nc.dram_tensor("ro", [DT, 128, VOCAB], WDT, kind="ExternalInput")
    cosT_d = nc.dram_tensor("cosT", [DT, 128, T], WDT, kind="ExternalInput")
    sinT_d = nc.dram_tensor("sinT", [DT, 128, T], WDT, kind="ExternalInput")
    ident_d = nc.dram_tensor("ident", [128, 128], MDT, kind="ExternalInput")
    logits_d = nc.dram_tensor("logits", [TT, 128, VOCAB], FP,
                              kind="ExternalOutput")

    groups = [[2 * i, 2 * i + 1] for i in range(n_cores // 2)]

    with tile.TileContext(nc) as tc:
        with (
            tc.tile_pool(name="pers", bufs=1) as pers,
            tc.tile_pool(name="wk", bufs=3) as wk,
            tc.tile_pool(name="lat", bufs=2) as latp,
            tc.tile_pool(name="sm", bufs=4) as sm,
            tc.tile_pool(name="col", bufs=6) as col,
            tc.tile_pool(name="ps", bufs=4, space="PSUM") as ps,
            tc.tile_pool(name="acc", bufs=2, space="PSUM") as acc,
            tc.tile_pool(name="dram", bufs=1, space="DRAM") as dram,
        ):
            env = dict(nc=nc, cfg=cfg, MDT=MDT, WDT=WDT, wk=wk, sm=sm, col=col,
                       ps=ps, acc=acc, latp=latp, groups=groups, eh_d=eh_d,
                       cosT_d=cosT_d, sinT_d=sinT_d, tc=tc)

            # ---------- persistent SBUF ----------
            ident = pers.tile([128, 128], MDT, tag="ident", name="ident")
            nc.sync.dma_start(ident[:], ident_d[:])
            env["ident"] = ident

            eps_col = pers.tile([128, 1], FP, tag="eps", name="eps_col")
            nc.vector.memset(eps_col[:], EPS)
            env["eps_col"] = eps_col

            idxf = pers.tile([1, T], FP, tag="idxf", name="idxf")
            nc.sync.dma_start(idxf[:], idxf_d[:])
            wte = []
            for i in range(VT):
                w = pers.tile([128, D], FP, tag=f"wte{i}", name=f"wte{i}")
                nc.sync.dma_start(w[:], wte_d[i])
                wte.append(w)

            env["dxh"] = dxh = []
            env["dyh"] = dyh = []
            for i in range(DT):
                dx = pers.tile([128, NH], WDT, tag=f"dxh{i}", name=f"dxh{i}")
                dy = pers.tile([128, NH], WDT, tag=f"dyh{i}", name=f"dyh{i}")
                nc.sync.dma_start(dx[:], dxh_d[i])
                nc.sync.dma_start(dy[:], dyh_d[i])
                dxh.append(dx)
                dyh.append(dy)

            ro = []
            for i in range(DT):
                r = pers.tile([128, VOCAB], WDT, tag=f"ro{i}", name=f"ro{i}")
                nc.sync.dma_start(r[:], ro_d[i])
                ro.append(r)

            if cfg.get("w_dt") == "bf16":
                env["ehs"] = ehs = []
                for m in range(NH // 128):
                    e = pers.tile([128, D], WDT, tag=f"ehs{m}", name=f"ehs{m}")
                    nc.sync.dma_start(e[:], eh_d[m])
                    ehs.append(e)
            else:
                env["ehs"] = None

            env["v_sb"] = v_sb = [
                pers.tile([128, D], MDT, tag=f"v{t}", name=f"v{t}")
                for t in range(TT)]
            env["vT"] = vT = [
                pers.tile([128, T], WDT, tag=f"vT{i}", name=f"vT{i}")
                for i in range(DT)]
            env["qT"] = [
                pers.tile([128, T], MDT, tag=f"qT{i}", name=f"qT{i}")
                for i in range(DT)]

            # ---------- embedding ----------
            lnwte = []
            for i in range(VT):
                lw = pers.tile([128, D], MDT, tag=f"lnwte{i}", name=f"lnwte{i}")
                _ln_rows(env, lw, wte[i], D)
                lnwte.append(lw)

            ones1 = pers.tile([1, 128], FP, tag="ones1", name="ones1")
            nc.vector.memset(ones1[:], 1.0)
            iotav = []
            for i in range(VT):
                iv = pers.tile([128, 1], FP, tag=f"iotav{i}", name=f"iotav{i}")
                nc.gpsimd.iota(iv[:], pattern=[[0, 1]], base=i * 128,
                               channel_multiplier=1,
                               allow_small_or_imprecise_dtypes=True)
                iotav.append(iv)

            # onehotT[v, t] = (idx[t] == v), built and consumed per 512-chunk
            with tc.tile_pool(name="emb", bufs=2) as embp:
                for c in range(T // 512):
                    cs = slice(c * 512, (c + 1) * 512)
                    pidx = ps.tile([128, 512], FP, tag="mm", name="pidx")
                    nc.tensor.matmul(pidx[:], ones1[:], idxf[:, cs],
                                     start=True, stop=True)
                    oh = []
                    for i in range(VT):
                        ohi = embp.tile([128, 512], MDT, tag="ohs", name="ohs")
                        nc.vector.tensor_scalar(ohi[:], pidx[:], iotav[i][:],
                                                None, op0=ALU.is_equal)
                        oh.append(ohi)
                    # v0 = LN(wte)[idx] for the 4 token tiles of this chunk
                    for tl in range(4):
                        t = c * 4 + tl
                        pv = ps.tile([128, D], FP, tag="mm", name="pv")
                        for i in range(VT):
                            nc.tensor.matmul(pv[:],
                                             (oh[i][:, tl * 128:(tl + 1) * 128]),
                                             (lnwte[i][:]),
                                             start=(i == 0), stop=(i == VT - 1))
                        nc.vector.tensor_copy(v_sb[t][:], pv[:])
                    for i in range(DT):
                        pvt = ps.tile([128, 512], FP, tag="mm", name="pvt")
                        for k in range(VT):
                            nc.tensor.matmul(
                                pvt[:],
                                (lnwte[k][:, i * 128:(i + 1) * 128]),
                                (oh[k][:]),
                                start=(k == 0), stop=(k == VT - 1))
                        nc.vector.tensor_copy(vT[i][:, cs], pvt[:])

            total_layers = cfg["reps"] * L
            if cfg["sched"] == "pipe":
                # ---------- pipelined layers: collectives overlap compute ----
                TH = T // 2
                env["up_lo"] = dram.tile([TH, D], FP, name="up_lo")
                env["up_hi"] = dram.tile([TH, D], FP, name="up_hi")
                env["upr_lo"] = dram.tile([TH, D], FP, name="upr_lo")
                env["upr_hi"] = dram.tile([TH, D], FP, name="upr_hi")
                for g in range(T // cfg["TCHUNK"]):
                    _emit_rope(env, g)
                for li in range(total_layers):
                    _emit_layer_pipe(env, li, total_layers)
                _emit_vnew_half(env, 1, rope=False, phase=2 * total_layers)
            else:
                env["up_dram"] = dram.tile([T, D], FP, name="upd")
                env["upr_dram"] = dram.tile([T, D], FP, name="uprd")
                for li in range(total_layers):
                    env["_layers_left"] = total_layers - 1 - li
                    _emit_layer(env)

            # ---------- readout ----------
            for t in range(TT):
                pl = ps.tile([128, VOCAB], FP, tag="mm", name="pl")
                for i in range(DT):
                    nc.tensor.matmul(pl[:],
                                     (vT[i][:, t * 128:(t + 1) * 128]),
                                     (ro[i][:]),
                                     start=(i == 0), stop=(i == DT - 1))
                lg = wk.tile([128, VOCAB], FP, tag="lg", name="lg")
                nc.vector.tensor_copy(lg[:], pl[:])
                nc.sync.dma_start(logits_d[t], lg[:])

    nc.compile()
    return nc


def _ln_rows(env, out_ap, in_ap, F, resid_ap=None):
    """LN over the free dim per partition row. If resid_ap: out = resid + ln(in)."""
    nc, sm, col = env["nc"], env["sm"], env["col"]
    st6 = col.tile([128, 6], FP, tag="bst", name="bst")
    nc.vector.bn_stats(st6[:], in_ap[:])
    st2 = col.tile([128, 2], FP, tag="bag", name="bag")
    nc.vector.bn_aggr(st2[:], st6[:])
    std = col.tile([128, 1], FP, tag="std", name="std")
    nc.scalar.activation(std[:], st2[:, 1:2], ACTF.Sqrt, bias=env["eps_col"][:])
    rstd = col.tile([128, 1], FP, tag="rstd", name="rstd")
    nc.vector.reciprocal(rstd[:], std[:])
    if resid_ap is None:
        nc.vector.tensor_scalar(out_ap[:], in_ap[:], st2[:, 0:1], rstd[:],
                                op0=ALU.subtract, op1=ALU.mult)
    else:
        tmp = sm.tile([128, F], FP, tag="lntmp", name="lntmp")
        nc.vector.tensor_scalar(tmp[:], in_ap[:], st2[:, 0:1], rstd[:],
                                op0=ALU.subtract, op1=ALU.mult)
        nc.vector.tensor_add(out_ap[:], resid_ap[:], tmp[:])


def _emit_rope(env, g):
    nc, WDT = env["nc"], env["WDT"]
    cfg = env["cfg"]
    TC = cfg["TCHUNK"]
    DT = cfg["D"] // 128
    wk, vT, qT = env["wk"], env["vT"], env["qT"]
    cosT_d, sinT_d = env["cosT_d"], env["sinT_d"]
    cs = slice(g * TC, (g + 1) * TC)
    for i in range(DT):
        o = 1 - i
        ctab = wk.tile([128, TC], WDT, tag="ctab", bufs=2, name="ctab")
        nc.sync.dma_start(ctab[:], cosT_d[i, :, cs])
        stab = wk.tile([128, TC], WDT, tag="stab", bufs=2, name="stab")
        nc.sync.dma_start(stab[:], sinT_d[i, :, cs])
        t1 = wk.tile([128, TC], FP, tag="rope", bufs=2, name="ropeA")
        nc.vector.tensor_mul(t1[:], vT[i][:, cs], ctab[:])
        t2 = wk.tile([128, TC], FP, tag="rope", bufs=2, name="ropeB")
        nc.vector.tensor_mul(t2[:], vT[o][:, cs], stab[:])
        if i == 0:
            nc.vector.tensor_sub(qT[i][:, cs], t1[:], t2[:])
        else:
            nc.vector.tensor_add(qT[i][:, cs], t1[:], t2[:])


def _emit_layer_pipe(env, li, total):
    """One layer with both AllReduces overlapped across the layer boundary.

    Emit order: chunks 0,1 -> AR_lo(l) -> vnew_hi(l-1) [consumes AR_hi(l-1)]
    -> chunks 2,3 -> AR_hi(l) -> vnew_lo(l) [consumes AR_lo(l)].
    Each collective gets ~2 chunks of compute to complete before its output
    is read, so in steady state no engine waits on it.
    """
    _emit_chunk(env, 0)
    _emit_chunk(env, 1)
    _fire_ar(env, 0)
    if li > 0:
        _emit_vnew_half(env, 1, rope=True, phase=2 * li)
    _emit_chunk(env, 2)
    _emit_chunk(env, 3)
    _fire_ar(env, 1)
    _emit_vnew_half(env, 0, rope=(li < total - 1), phase=2 * li + 1)


def _fire_ar(env, half):
    nc, cfg = env["nc"], env["cfg"]
    src = env["up_hi"] if half else env["up_lo"]
    dst = env["upr_hi"] if half else env["upr_lo"]
    if cfg.get("no_cc"):
        nc.sync.dma_start(dst[:], src[:])
    else:
        nc.gpsimd.collective_compute(
            "AllReduce", ALU.add, replica_groups=env["groups"],
            ins=[src[:].opt()], outs=[dst[:].opt()])


def _emit_vnew_half(env, half, rope=True, phase=0):
    nc, MDT = env["nc"], env["MDT"]
    cfg = env["cfg"]
    D, T = cfg["D"], cfg["T"]
    DT = D // 128
    TT = T // 128
    sm, ps = env["sm"], env["ps"]
    v_sb, vT, ident = env["v_sb"], env["vT"], env["ident"]
    upr = env["upr_hi"] if half else env["upr_lo"]
    t_lo = half * (TT // 2)
    for t in range(t_lo, t_lo + TT // 2):
        r = (t - t_lo) * 128
        u = sm.tile([128, D], FP, tag="upr", name="upr")
        # Logical-time hint: the scheduler's internal sim treats collectives
        # as cheap and would hoist this AllReduce-dependent load ahead of
        # independent DMAs in the same queue, head-of-line blocking them
        # behind the collective. Force it to schedule after this phase.
        with env["tc"].tile_wait_until(ms=phase + 1):
            nc.sync.dma_start(u[:], upr[r:r + 128])
        _ln_rows(env, v_sb[t], u, D, resid_ap=v_sb[t])
        for i in range(DT):
            ptr = ps.tile([128, 128], MDT, tag="mm", name="ptr2")
            nc.tensor.matmul(ptr[:], v_sb[t][:, i * 128:(i + 1) * 128],
                             ident[:], is_transpose=True, start=True, stop=True)
            nc.vector.tensor_copy(vT[i][:, t * 128:(t + 1) * 128], ptr[:])
    if rope:
        _emit_rope(env, 2 * half)
        _emit_rope(env, 2 * half + 1)


def _emit_chunk(env, g):
    """Attention + MLP for one 512-token chunk; partial update rows written
    to up_lo (chunks 0,1) / up_hi (chunks 2,3)."""
    nc, cfg, MDT = env["nc"], env["cfg"], env["MDT"]
    WDT = env["WDT"]
    D = cfg["D"]
    NH = cfg["N"] // 2
    TC = cfg["TCHUNK"]
    MC = NH // 128
    NSUB = TC // 128
    DT = D // 128
    wk, sm, ps, acc = env["wk"], env["sm"], env["ps"], env["acc"]
    vT, qT, v_sb, latp = env["vT"], env["qT"], env["v_sb"], env["latp"]
    dxh, dyh = env["dxh"], env["dyh"]
    ident, eh_d = env["ident"], env["eh_d"]
    up_d = env["up_hi"] if g >= 2 else env["up_lo"]
    up_r0 = (g - 2) * TC if g >= 2 else g * TC
    XPRE = 3

    t0 = g * TC
    nkb = (g + 1) * NSUB

    def emit_x(m, xrs):
        x_ps = ps.tile([128, TC], FP, tag="mm", name="x_ps")
        for i in range(DT):
            nc.tensor.matmul(x_ps[:],
                             dxh[i][:, m * 128:(m + 1) * 128],
                             vT[i][:, t0:t0 + TC],
                             start=(i == 0), stop=(i == DT - 1))
        xr = wk.tile([128, TC], FP, tag="xr", bufs=4, name="xr")
        nc.scalar.activation(xr[:], x_ps[:], ACTF.Relu)
        xrs[m] = xr

    def emit_av(sb, e_sb, aT_ps):
        for i in range(DT):
            nc.tensor.matmul(aT_ps[:, i, :],
                             v_sb[sb][:, i * 128:(i + 1) * 128],
                             e_sb[:],
                             start=(sb == 0), stop=(sb == nkb - 1),
                             skip_group_check=True)

    # ---- attention: energyT[s, t] blocks, AV lags two blocks
    aT_ps = acc.tile([128, DT, TC], FP, tag="acc", name="aT_ps")
    pend = []
    for sb in range(nkb):
        e_ps = ps.tile([128, TC], FP, tag="mm", name="e_ps")
        for i in range(DT):
            nc.tensor.matmul(e_ps[:],
                             qT[i][:, sb * 128:(sb + 1) * 128],
                             qT[i][:, t0:t0 + TC],
                             start=(i == 0), stop=(i == DT - 1))
        e_sb = wk.tile([128, TC], MDT, tag="esb", bufs=3, name="e_sb")
        nc.vector.tensor_copy(e_sb[:], e_ps[:])
        diag_j = sb - (nkb - NSUB)
        if diag_j >= 0:
            nc.gpsimd.affine_select(e_sb[:], e_sb[:], pattern=[[1, TC]],
                                    compare_op=ALU.is_ge, fill=0.0,
                                    base=-128 * diag_j,
                                    channel_multiplier=-1)
        pend.append((sb, e_sb))
        if len(pend) > 2:
            s0, e0 = pend.pop(0)
            emit_av(s0, e0, aT_ps)
    xrs = {}
    emit_x(0, xrs)
    for s0, e0 in pend:
        emit_av(s0, e0, aT_ps)
    emit_x(1, xrs)
    # ---- aT -> a (PE transposes), LN(a), la -> laT slices ----
    aT_sb = [wk.tile([128, TC], MDT, tag=f"aTs{i}", bufs=2, name="aT_sb")
             for i in range(DT)]
    for i in range(DT):
        nc.scalar.copy(aT_sb[i][:], aT_ps[:, i, :])
    emit_x(2, xrs)
    laT = [latp.tile([128, TC], WDT, tag=f"laTs{i}", name=f"laT{i}")
           for i in range(DT)]
    for tsub in range(NSUB):
        a_ti = ps.tile([128, D], MDT, tag="mm", name="a_ti")
        for i in range(DT):
            nc.tensor.matmul(a_ti[:, i * 128:(i + 1) * 128],
                             aT_sb[i][:, tsub * 128:(tsub + 1) * 128],
                             ident[:], is_transpose=True,
                             start=(i == 0), stop=(i == DT - 1),
                             skip_group_check=True)
        la = sm.tile([128, D], MDT, tag="la", name="la")
        _ln_rows(env, la, a_ti, D)
        for i in range(DT):
            ptr = ps.tile([128, 128], MDT, tag="mm", name="ptr")
            nc.tensor.matmul(ptr[:], la[:, i * 128:(i + 1) * 128], ident[:],
                             is_transpose=True, start=True, stop=True)
            nc.vector.tensor_copy(
                laT[i][:, tsub * 128:(tsub + 1) * 128], ptr[:])
    # ---- MLP: y = relu(Dy^T laT) * x, updateT += Eh^T yel ----
    upT_ps = acc.tile([128, DT, TC], FP, tag="acc", name="upT_ps")

    def emit_up(m, yel, ehm):
        for i in range(DT):
            nc.tensor.matmul(upT_ps[:, i, :],
                             ehm[:, i * 128:(i + 1) * 128],
                             yel[:],
                             start=(m == 0), stop=(m == MC - 1),
                             skip_group_check=True)

    pup = None
    for m in range(MC):
        if env["ehs"] is not None:
            ehm = env["ehs"][m]
        else:
            ehm = wk.tile([128, D], WDT, tag="ehst", bufs=4, name="ehm")
            nc.sync.dma_start(ehm[:], eh_d[m])
        if m + XPRE < MC:
            emit_x(m + XPRE, xrs)
        y_ps = ps.tile([128, TC], FP, tag="mm", name="y_ps")
        for i in range(DT):
            nc.tensor.matmul(y_ps[:],
                             dyh[i][:, m * 128:(m + 1) * 128],
                             laT[i][:],
                             start=(i == 0), stop=(i == DT - 1))
        yr = wk.tile([128, TC], FP, tag="yr", bufs=2, name="yr")
        nc.scalar.activation(yr[:], y_ps[:], ACTF.Relu)
        yel = wk.tile([128, TC], WDT, tag="yel", bufs=2, name="yel")
        nc.vector.tensor_mul(yel[:], yr[:], xrs.pop(m)[:])
        if pup is not None:
            emit_up(*pup)
        pup = (m, yel, ehm)
    emit_up(*pup)
    # ---- upT -> up (PE transposes), evacuate chunk rows ----
    upT_sb = [wk.tile([128, TC], MDT, tag=f"uTs{i}", bufs=1, name="upT_sb")
              for i in range(DT)]
    for i in range(DT):
        nc.scalar.copy(upT_sb[i][:], upT_ps[:, i, :])
    for tsub in range(NSUB):
        u_ti = ps.tile([128, D], MDT, tag="mm", name="u_ti")
        for i in range(DT):
            nc.tensor.matmul(u_ti[:, i * 128:(i + 1) * 128],
                             upT_sb[i][:, tsub * 128:(tsub + 1) * 128],
                             ident[:], is_transpose=True,
                             start=(i == 0), stop=(i == DT - 1),
                             skip_group_check=True)
        u_sb = wk.tile([128, D], FP, tag="usb", bufs=2, name="u_sb")
        nc.vector.tensor_copy(u_sb[:], u_ti[:])
        r0 = up_r0 + tsub * 128
        nc.sync.dma_start(up_d[r0:r0 + 128], u_sb[:])


def _emit_layer(env):
    nc, cfg, MDT = env["nc"], env["cfg"], env["MDT"]
    WDT = env["WDT"]
    T, D = cfg["T"], cfg["D"]
    NH = cfg["N"] // 2
    TC = cfg["TCHUNK"]
    G = T // TC
    TT = T // 128
    MC = NH // 128
    NSUB = TC // 128
    DT = D // 128
    wk, sm, col, ps, acc = env["wk"], env["sm"], env["col"], env["ps"], env["acc"]
    vT, qT, v_sb, latp = env["vT"], env["qT"], env["v_sb"], env["latp"]
    dxh, dyh = env["dxh"], env["dyh"]
    ident, eh_d = env["ident"], env["eh_d"]
    cosT_d, sinT_d = env["cosT_d"], env["sinT_d"]
    up_dram, upr_dram = env["up_dram"], env["upr_dram"]
    XPRE = 3                              # x m-chunks emitted ahead of y

    def emit_x(m, t0, xrs):
        x_ps = ps.tile([128, TC], FP, tag="mm", name="x_ps")
        for i in range(DT):
            nc.tensor.matmul(x_ps[:],
                             dxh[i][:, m * 128:(m + 1) * 128],
                             vT[i][:, t0:t0 + TC],
                             start=(i == 0), stop=(i == DT - 1))
        xr = wk.tile([128, TC], FP, tag="xr", bufs=4, name="xr")
        nc.scalar.activation(xr[:], x_ps[:], ACTF.Relu)
        xrs[m] = xr

    def emit_av(sb, e_sb, aT_ps, nkb):
        # aT[d, t] += v[s, d]^T e[s, t]  (free dim TC keeps f32r at full rate)
        for i in range(DT):
            nc.tensor.matmul(aT_ps[:, i, :],
                             v_sb[sb][:, i * 128:(i + 1) * 128],
                             e_sb[:],
                             start=(sb == 0), stop=(sb == nkb - 1),
                             skip_group_check=True)

    for g in range(G):
        t0 = g * TC
        nkb = (g + 1) * NSUB              # causal key-block count
        # ---- RoPE for this chunk (chunk 0 may have been emitted by prev layer)
        if not (g == 0 and env.pop("_rope0_done", False)):
            _emit_rope(env, g)
        # ---- attention: energyT[s, t] blocks, AV lags two blocks (PE pipelining)
        aT_ps = acc.tile([128, DT, TC], FP, tag="acc", name="aT_ps")
        pend = []
        for sb in range(nkb):
            e_ps = ps.tile([128, TC], FP, tag="mm", name="e_ps")
            for i in range(DT):
                nc.tensor.matmul(e_ps[:],
                                 qT[i][:, sb * 128:(sb + 1) * 128],
                                 qT[i][:, t0:t0 + TC],
                                 start=(i == 0), stop=(i == DT - 1))
            e_sb = wk.tile([128, TC], MDT, tag="esb", bufs=3, name="e_sb")
            nc.vector.tensor_copy(e_sb[:], e_ps[:])
            diag_j = sb - (nkb - NSUB)
            if diag_j >= 0:
                # causal mask: keep where t - s - 128*j >= 0
                nc.gpsimd.affine_select(e_sb[:], e_sb[:], pattern=[[1, TC]],
                                        compare_op=ALU.is_ge, fill=0.0,
                                        base=-128 * diag_j,
                                        channel_multiplier=-1)
            pend.append((sb, e_sb))
            if len(pend) > 2:
                s0, e0 = pend.pop(0)
                emit_av(s0, e0, aT_ps, nkb)
        xrs = {}
        emit_x(0, t0, xrs)
        for s0, e0 in pend:
            emit_av(s0, e0, aT_ps, nkb)
        pend = []
        emit_x(1, t0, xrs)
        # ---- aT -> a (PE transposes), LN(a), la -> laT slices ----
        aT_sb = [wk.tile([128, TC], MDT, tag=f"aTs{i}", bufs=2, name="aT_sb")
                 for i in range(DT)]
        for i in range(DT):
            nc.scalar.copy(aT_sb[i][:], aT_ps[:, i, :])
        emit_x(2, t0, xrs)
        laT = [latp.tile([128, TC], WDT, tag=f"laTs{i}", name=f"laT{i}")
               for i in range(DT)]
        for tsub in range(NSUB):
            a_ti = ps.tile([128, D], MDT, tag="mm", name="a_ti")
            for i in range(DT):
                nc.tensor.matmul(a_ti[:, i * 128:(i + 1) * 128],
                                 aT_sb[i][:, tsub * 128:(tsub + 1) * 128],
                                 ident[:], is_transpose=True,
                                 start=(i == 0), stop=(i == DT - 1),
                                 skip_group_check=True)
            la = sm.tile([128, D], MDT, tag="la", name="la")
            _ln_rows(env, la, a_ti, D)
            for i in range(DT):
                ptr = ps.tile([128, 128], MDT, tag="mm", name="ptr")
                nc.tensor.matmul(ptr[:], la[:, i * 128:(i + 1) * 128], ident[:],
                                 is_transpose=True, start=True, stop=True)
                nc.vector.tensor_copy(
                    laT[i][:, tsub * 128:(tsub + 1) * 128], ptr[:])
        # ---- MLP: y = relu(Dy^T laT) * x, updateT += Eh^T yel, x pipelined ahead
        upT_ps = acc.tile([128, DT, TC], FP, tag="acc", name="upT_ps")

        def emit_up(m, yel, ehm):
            for i in range(DT):
                nc.tensor.matmul(upT_ps[:, i, :],
                                 ehm[:, i * 128:(i + 1) * 128],
                                 yel[:],
                                 start=(m == 0), stop=(m == MC - 1),
                                 skip_group_check=True)

        pup = None
        for m in range(MC):
            if env["ehs"] is not None:
                ehm = env["ehs"][m]
            else:
                ehm = wk.tile([128, D], WDT, tag="ehst", bufs=4, name="ehm")
                nc.sync.dma_start(ehm[:], eh_d[m])
            if m + XPRE < MC:
                emit_x(m + XPRE, t0, xrs)
            y_ps = ps.tile([128, TC], FP, tag="mm", name="y_ps")
            for i in range(DT):
                nc.tensor.matmul(y_ps[:],
                                 dyh[i][:, m * 128:(m + 1) * 128],
                                 laT[i][:],
                                 start=(i == 0), stop=(i == DT - 1))
            yr = wk.tile([128, TC], FP, tag="yr", bufs=2, name="yr")
            nc.scalar.activation(yr[:], y_ps[:], ACTF.Relu)
            yel = wk.tile([128, TC], WDT, tag="yel", bufs=2, name="yel")
            nc.vector.tensor_mul(yel[:], yr[:], xrs.pop(m)[:])
            if pup is not None:
                emit_up(*pup)
            pup = (m, yel, ehm)
        emit_up(*pup)
        # ---- upT -> up (PE transposes), evacuate chunk to DRAM ----
        upT_sb = [wk.tile([128, TC], MDT, tag=f"uTs{i}", bufs=1, name="upT_sb")
                  for i in range(DT)]
        for i in range(DT):
            nc.scalar.copy(upT_sb[i][:], upT_ps[:, i, :])
        for tsub in range(NSUB):
            u_ti = ps.tile([128, D], MDT, tag="mm", name="u_ti")
            for i in range(DT):
                nc.tensor.matmul(u_ti[:, i * 128:(i + 1) * 128],
                                 upT_sb[i][:, tsub * 128:(tsub + 1) * 128],
                                 ident[:], is_transpose=True,
                                 start=(i == 0), stop=(i == DT - 1),
                                 skip_group_check=True)
            u_sb = wk.tile([128, D], FP, tag="usb", bufs=2, name="u_sb")
            nc.vector.tensor_copy(u_sb[:], u_ti[:])
            r0 = t0 + tsub * 128
            nc.sync.dma_start(up_dram[r0:r0 + 128], u_sb[:])
        # ---- AllReduce each completed half so it overlaps later chunks
        if g % 2 == 1 or g == G - 1:
            hs = slice(env.get("_ar_start", 0), (g + 1) * TC)
            env["_ar_start"] = (g + 1) * TC if g < G - 1 else 0
            if cfg.get("no_cc"):
                nc.sync.dma_start(upr_dram[hs], up_dram[hs])
            else:
                nc.gpsimd.collective_compute(
                    "AllReduce", ALU.add, replica_groups=env["groups"],
                    ins=[up_dram[hs].opt()], outs=[upr_dram[hs].opt()])

    # ---- v += LN(update); refresh vT; next layer's chunk-0 rope between halves
    if G > 1:
        nfirst = TC // 128            # rope(0) needs only vT tiles of chunk 0
        _emit_vnew(env, 0, nfirst)
        if env.get("_layers_left", 0) > 0:
            _emit_rope(env, 0)
            env["_rope0_done"] = True
        _emit_vnew(env, nfirst, TT)
    else:
        _emit_vnew(env, 0, TT)


def _emit_vnew(env, t_lo, t_hi):
    nc, MDT = env["nc"], env["MDT"]
    D = env["cfg"]["D"]
    DT = D // 128
    sm, ps = env["sm"], env["ps"]
    v_sb, vT, ident = env["v_sb"], env["vT"], env["ident"]
    upr_dram = env["upr_dram"]
    for t in range(t_lo, t_hi):
        upr = sm.tile([128, D], FP, tag="upr", name="upr")
        nc.sync.dma_start(upr[:], upr_dram[t * 128:(t + 1) * 128])
        _ln_rows(env, v_sb[t], upr, D, resid_ap=v_sb[t])
        for i in range(DT):
            ptr = ps.tile([128, 128], MDT, tag="mm", name="ptr2")
            nc.tensor.matmul(ptr[:], v_sb[t][:, i * 128:(i + 1) * 128], ident[:],
                             is_transpose=True, start=True, stop=True)
            nc.vector.tensor_copy(vT[i][:, t * 128:(t + 1) * 128], ptr[:])


# ====================== host side ======================

_BUILD_CACHE = {}


def shard_inputs(cfg, idx, wte, encoder, decoder_x, decoder_y, readout):
    """Returns the list of per-core input dicts."""
    import ml_dtypes
    wnp = ml_dtypes.bfloat16 if cfg.get("w_dt") == "bf16" else np.float32
    T, D, VOCAB = cfg["T"], cfg["D"], cfg["VOCAB"]
    NH = cfg["N"] // 2
    DT = D // 128
    VT = VOCAB // 128
    heads_per_half = cfg["H"] // 2

    inv_freq = 1.0 / (10000.0 ** (np.arange(0, D, 2, dtype=np.float64) / D))
    tpos = np.arange(T, dtype=np.float64)
    freqs = np.outer(tpos, inv_freq)
    emb = np.concatenate([freqs, freqs], axis=-1)     # [T, D]
    cosT = np.ascontiguousarray(np.cos(emb).T.astype(wnp)).reshape(DT, 128, T)
    sinT = np.ascontiguousarray(np.sin(emb).T.astype(wnp)).reshape(DT, 128, T)
    ident = np.eye(128, dtype=np.float32)

    wte_s = np.ascontiguousarray(wte.astype(np.float32)).reshape(VT, 128, D)
    ro_s = np.ascontiguousarray(readout.astype(wnp)).reshape(DT, 128, VOCAB)

    in_maps = []
    for c in range(cfg["n_cores"]):
        b, h = c // 2, c % 2
        heads = range(h * heads_per_half, (h + 1) * heads_per_half)
        dxh = np.concatenate([decoder_x[hh] for hh in heads], axis=-1)  # [D, NH]
        dyh = np.concatenate([decoder_y[hh] for hh in heads], axis=-1)
        ehh = encoder[h * NH:(h + 1) * NH]                              # [NH, D]
        in_maps.append(dict(
            idxf=np.ascontiguousarray(idx[b].astype(np.float32)[None, :]),
            wte=wte_s,
            dxh=np.ascontiguousarray(dxh.astype(wnp)).reshape(DT, 128, NH),
            dyh=np.ascontiguousarray(dyh.astype(wnp)).reshape(DT, 128, NH),
            eh=np.ascontiguousarray(ehh.astype(wnp)).reshape(NH // 128, 128, D),
            ro=ro_s,
            cosT=cosT, sinT=sinT, ident=ident,
        ))
    return in_maps


def get_program(cfg):
    key = (cfg["T"], cfg["N"], cfg["L"], cfg["mm_dt"], cfg.get("w_dt"),
           cfg["reps"], cfg["n_cores"], cfg["TCHUNK"], cfg.get("no_cc"))
    if key not in _BUILD_CACHE:
        _BUILD_CACHE[key] = build_program(cfg)
    return _BUILD_CACHE[key]


def kernel(idx, wte, encoder, decoder_x, decoder_y, readout):
    cfg = default_cfg()
    nc = get_program(cfg)
    in_maps = shard_inputs(cfg, np.asarray(idx), np.asarray(wte),
                           np.asarray(encoder), np.asarray(decoder_x),
                           np.asarray(decoder_y), np.asarray(readout))
    res = run_bass_kernel_spmd(nc, in_maps, list(range(cfg["n_cores"])))
    B, T, VOCAB = cfg["B"], cfg["T"], cfg["VOCAB"]
    out = np.empty((B, T, VOCAB), np.float32)
    for b in range(B):
        out[b] = res.results[2 * b]["logits"].reshape(T, VOCAB)
    return out



# revision 13
# speedup vs baseline: 2020.7906x; 4.6300x over previous
"""Trainium2 Bass kernel for the BDH-style sparse-attention network.

Reference computation (per batch b, all fp32):
  v = LN(wte[idx])                                   [T, D]
  repeat L times:
    x   = relu(v @ Dx)                               [T, N]   (Dx: [D, N] = decoder_x heads concat)
    a   = causal_linear_attention(v) (RoPE, no softmax, tril mask)
    y   = relu(LN(a) @ Dy) * x                       [T, N]
    v   = v + LN(y @ E)                              [T, D]   (E: [N, D] = encoder)
  logits = v @ readout                               [T, VOCAB]

Sharding over 8 NeuronCores: core c -> batch b = c//2, neuron half h = c%2.
Each core holds half the neuron dim (N/2 columns of Dx/Dy, N/2 rows of E) and
computes the full attention for its batch; the partial `y @ E` update is
summed with an AllReduce over core pairs [[0,1],[2,3],[4,5],[6,7]].

On-device layout: token-major tiles v [128t, D] plus a transposed copy
vT [128d, T] maintained via PE transposes, so every matmul has its
contraction dim on partitions without extra data movement.
"""

import numpy as np

import concourse.bass as bass
import concourse.bacc as bacc
import concourse.mybir as mybir
import concourse.tile as tile
from concourse.bass_utils import run_bass_kernel_spmd

FP = mybir.dt.float32
AX = mybir.AxisListType
ALU = mybir.AluOpType
ACTF = mybir.ActivationFunctionType
EPS = 1e-5


def default_cfg():
    return dict(
        T=2048, D=256, N=8192, H=4, VOCAB=256, L=6, B=4,
        TCHUNK=512,          # tokens per chunk == attention query block
        mm_dt="f32r",        # "f32r" | "f32" : dtype view fed to the PE
        w_dt="mm",           # "bf16" | "mm" : matmul dtype for the MLP path
        sched="pipe",        # "pipe" | "orig": collective-overlap pipeline
        n_cores=8,
        reps=1,              # layer-stack repeats (for wall-clock timing deltas)
    )


def build_program(cfg):
    """Builds and compiles the per-core SPMD bass program."""
    T, D, VOCAB, L = cfg["T"], cfg["D"], cfg["VOCAB"], cfg["L"]
    NH = cfg["N"] // 2
    TC = cfg["TCHUNK"]
    TT = T // 128
    DT = D // 128
    VT = VOCAB // 128
    n_cores = cfg["n_cores"]
    assert D == 256 and TC % 128 == 0 and T % TC == 0 and T % 512 == 0

    MDT = mybir.dt.float32r if cfg["mm_dt"] == "f32r" else FP
    WDT = mybir.dt.bfloat16 if cfg.get("w_dt") == "bf16" else MDT

    nc = bacc.Bacc("TRN2", target_bir_lowering=False, debug=False,
                   num_devices=n_cores)

    idxf_d = nc.dram_tensor("idxf", [1, T], FP, kind="ExternalInput")
    wte_d = nc.dram_tensor("wte", [VT, 128, D], FP, kind="ExternalInput")
    dxh_d = nc.dram_tensor("dxh", [DT, 128, NH], WDT, kind="ExternalInput")
    dyh_d = nc.dram_tensor("dyh", [DT, 128, NH], WDT, kind="ExternalInput")
    eh_d = nc.dram_tensor("eh", [NH // 128, 128, D], WDT, kind="ExternalInput")
    ro_d = nc.dram_tensor("ro", [DT, 128, VOCAB], WDT, kind="ExternalInput")
    cosT_d = nc.dram_tensor("cosT", [DT, 128, T], WDT, kind="ExternalInput")
    sinT_d = nc.dram_tensor("sinT", [DT, 128, T], WDT, kind="ExternalInput")
    ident_d = nc.dram_tensor("ident", [128, 128], MDT, kind="ExternalInput")
    logits_d = nc.dram_tensor("logits", [TT, 128, VOCAB], FP,
                              kind="ExternalOutput")

    groups = [[2 * i, 2 * i + 1] for i in range(n_cores // 2)]

    with tile.TileContext(nc) as tc:
        with (
            tc.tile_pool(name="pers", bufs=1) as pers,
            tc.tile_pool(name="wk", bufs=3) as wk,
            tc.tile_pool(name="lat", bufs=2) as latp,
            tc.tile_pool(name="sm", bufs=4) as sm,
            tc.tile_pool(name="col", bufs=6) as col,
            tc.tile_pool(name="ps", bufs=4, space="PSUM") as ps,
            tc.tile_pool(name="acc", bufs=2, space="PSUM") as acc,
            tc.tile_pool(name="dram", bufs=1, space="DRAM") as dram,
        ):
            env = dict(nc=nc, cfg=cfg, MDT=MDT, WDT=WDT, wk=wk, sm=sm, col=col,
                       ps=ps, acc=acc, latp=latp, groups=groups, eh_d=eh_d,
                       cosT_d=cosT_d, sinT_d=sinT_d, tc=tc)

            # ---------- persistent SBUF ----------
            ident = pers.tile([128, 128], MDT, tag="ident", name="ident")
            nc.sync.dma_start(ident[:], ident_d[:])
            env["ident"] = ident

            eps_col = pers.tile([128, 1], FP, tag="eps", name="eps_col")
            nc.vector.memset(eps_col[:], EPS)
            env["eps_col"] = eps_col

            idxf = pers.tile([1, T], FP, tag="idxf", name="idxf")
            nc.sync.dma_start(idxf[:], idxf_d[:])
            wte = []
            for i in range(VT):
                w = pers.tile([128, D], FP, tag=f"wte{i}", name=f"wte{i}")
                nc.sync.dma_start(w[:], wte_d[i])
                wte.append(w)

            env["dxh"] = dxh = []
            env["dyh"] = dyh = []
            for i in range(DT):
                dx = pers.tile([128, NH], WDT, tag=f"dxh{i}", name=f"dxh{i}")
                dy = pers.tile([128, NH], WDT, tag=f"dyh{i}", name=f"dyh{i}")
                nc.sync.dma_start(dx[:], dxh_d[i])
                nc.sync.dma_start(dy[:], dyh_d[i])
                dxh.append(dx)
                dyh.append(dy)

            ro = []
            for i in range(DT):
                r = pers.tile([128, VOCAB], WDT, tag=f"ro{i}", name=f"ro{i}")
                nc.sync.dma_start(r[:], ro_d[i])
                ro.append(r)

            if cfg.get("w_dt") == "bf16":
                env["ehs"] = ehs = []
                for m in range(NH // 128):
                    e = pers.tile([128, D], WDT, tag=f"ehs{m}", name=f"ehs{m}")
                    nc.sync.dma_start(e[:], eh_d[m])
                    ehs.append(e)
            else:
                env["ehs"] = None

            env["v_sb"] = v_sb = [
                pers.tile([128, D], MDT, tag=f"v{t}", name=f"v{t}")
                for t in range(TT)]
            env["vT"] = vT = [
                pers.tile([128, T], WDT, tag=f"vT{i}", name=f"vT{i}")
                for i in range(DT)]
            env["qT"] = [
                pers.tile([128, T], MDT, tag=f"qT{i}", name=f"qT{i}")
                for i in range(DT)]

            # ---------- embedding ----------
            lnwte = []
            for i in range(VT):
                lw = pers.tile([128, D], MDT, tag=f"lnwte{i}", name=f"lnwte{i}")
                _ln_rows(env, lw, wte[i], D)
                lnwte.append(lw)

            ones1 = pers.tile([1, 128], FP, tag="ones1", name="ones1")
            nc.vector.memset(ones1[:], 1.0)
            iotav = []
            for i in range(VT):
                iv = pers.tile([128, 1], FP, tag=f"iotav{i}", name=f"iotav{i}")
                nc.gpsimd.iota(iv[:], pattern=[[0, 1]], base=i * 128,
                               channel_multiplier=1,
                               allow_small_or_imprecise_dtypes=True)
                iotav.append(iv)

            # onehotT[v, t] = (idx[t] == v), built and consumed per 512-chunk
            with tc.tile_pool(name="emb", bufs=2) as embp:
                for c in range(T // 512):
                    cs = slice(c * 512, (c + 1) * 512)
                    pidx = ps.tile([128, 512], FP, tag="mm", name="pidx")
                    nc.tensor.matmul(pidx[:], ones1[:], idxf[:, cs],
                                     start=True, stop=True)
                    oh = []
                    for i in range(VT):
                        ohi = embp.tile([128, 512], MDT, tag="ohs", name="ohs")
                        nc.vector.tensor_scalar(ohi[:], pidx[:], iotav[i][:],
                                                None, op0=ALU.is_equal)
                        oh.append(ohi)
                    # v0 = LN(wte)[idx] for the 4 token tiles of this chunk
                    for tl in range(4):
                        t = c * 4 + tl
                        pv = ps.tile([128, D], FP, tag="mm", name="pv")
                        for i in range(VT):
                            nc.tensor.matmul(pv[:],
                                             (oh[i][:, tl * 128:(tl + 1) * 128]),
                                             (lnwte[i][:]),
                                             start=(i == 0), stop=(i == VT - 1))
                        nc.vector.tensor_copy(v_sb[t][:], pv[:])
                    for i in range(DT):
                        pvt = ps.tile([128, 512], FP, tag="mm", name="pvt")
                        for k in range(VT):
                            nc.tensor.matmul(
                                pvt[:],
                                (lnwte[k][:, i * 128:(i + 1) * 128]),
                                (oh[k][:]),
                                start=(k == 0), stop=(k == VT - 1))
                        nc.vector.tensor_copy(vT[i][:, cs], pvt[:])

            total_layers = cfg["reps"] * L
            if cfg["sched"] == "pipe":
                # ---------- pipelined layers: collectives overlap compute ----
                TH = T // 2
                env["up_lo"] = dram.tile([TH, D], FP, name="up_lo")
                env["up_hi"] = dram.tile([TH, D], FP, name="up_hi")
                env["upr_lo"] = dram.tile([TH, D], FP, name="upr_lo")
                env["upr_hi"] = dram.tile([TH, D], FP, name="upr_hi")
                for g in range(T // cfg["TCHUNK"]):
                    _emit_rope(env, g)
                for li in range(total_layers):
                    _emit_layer_pipe(env, li, total_layers)
                _emit_vnew_half(env, 1, rope=False, phase=2 * total_layers)
            else:
                env["up_dram"] = dram.tile([T, D], FP, name="upd")
                env["upr_dram"] = dram.tile([T, D], FP, name="uprd")
                for li in range(total_layers):
                    env["_layers_left"] = total_layers - 1 - li
                    _emit_layer(env)

            # ---------- readout ----------
            for t in range(TT):
                pl = ps.tile([128, VOCAB], FP, tag="mm", name="pl")
                for i in range(DT):
                    nc.tensor.matmul(pl[:],
                                     (vT[i][:, t * 128:(t + 1) * 128]),
                                     (ro[i][:]),
                                     start=(i == 0), stop=(i == DT - 1))
                lg = wk.tile([128, VOCAB], FP, tag="lg", name="lg")
                nc.vector.tensor_copy(lg[:], pl[:])
                nc.sync.dma_start(logits_d[t], lg[:])

    nc.compile()
    return nc


def _ln_rows(env, out_ap, in_ap, F, resid_ap=None):
    """LN over the free dim per partition row. If resid_ap: out = resid + ln(in)."""
    nc, sm, col = env["nc"], env["sm"], env["col"]
    st6 = col.tile([128, 6], FP, tag="bst", name="bst")
    nc.vector.bn_stats(st6[:], in_ap[:])
    st2 = col.tile([128, 2], FP, tag="bag", name="bag")
    nc.vector.bn_aggr(st2[:], st6[:])
    std = col.tile([128, 1], FP, tag="std", name="std")
    nc.scalar.activation(std[:], st2[:, 1:2], ACTF.Sqrt, bias=env["eps_col"][:])
    rstd = col.tile([128, 1], FP, tag="rstd", name="rstd")
    nc.vector.reciprocal(rstd[:], std[:])
    if resid_ap is None:
        nc.vector.tensor_scalar(out_ap[:], in_ap[:], st2[:, 0:1], rstd[:],
                                op0=ALU.subtract, op1=ALU.mult)
    else:
        tmp = sm.tile([128, F], FP, tag="lntmp", name="lntmp")
        nc.vector.tensor_scalar(tmp[:], in_ap[:], st2[:, 0:1], rstd[:],
                                op0=ALU.subtract, op1=ALU.mult)
        nc.vector.tensor_add(out_ap[:], resid_ap[:], tmp[:])


def _emit_rope(env, g):
    nc, WDT = env["nc"], env["WDT"]
    cfg = env["cfg"]
    TC = cfg["TCHUNK"]
    DT = cfg["D"] // 128
    wk, vT, qT = env["wk"], env["vT"], env["qT"]
    cosT_d, sinT_d = env["cosT_d"], env["sinT_d"]
    cs = slice(g * TC, (g + 1) * TC)
    for i in range(DT):
        o = 1 - i
        ctab = wk.tile([128, TC], WDT, tag="ctab", bufs=2, name="ctab")
        nc.sync.dma_start(ctab[:], cosT_d[i, :, cs])
        stab = wk.tile([128, TC], WDT, tag="stab", bufs=2, name="stab")
        nc.sync.dma_start(stab[:], sinT_d[i, :, cs])
        t1 = wk.tile([128, TC], FP, tag="rope", bufs=2, name="ropeA")
        nc.vector.tensor_mul(t1[:], vT[i][:, cs], ctab[:])
        t2 = wk.tile([128, TC], FP, tag="rope", bufs=2, name="ropeB")
        nc.vector.tensor_mul(t2[:], vT[o][:, cs], stab[:])
        if i == 0:
            nc.vector.tensor_sub(qT[i][:, cs], t1[:], t2[:])
        else:
            nc.vector.tensor_add(qT[i][:, cs], t1[:], t2[:])


def _emit_layer_pipe(env, li, total):
    """One layer with both AllReduces overlapped across the layer boundary.

    Emit order: chunks 0,1 -> AR_lo(l) -> vnew_hi(l-1) [consumes AR_hi(l-1)]
    -> chunks 2,3 -> AR_hi(l) -> vnew_lo(l) [consumes AR_lo(l)].
    Each collective gets ~2 chunks of compute to complete before its output
    is read, so in steady state no engine waits on it.
    """
    _emit_chunk(env, 0)
    _emit_chunk(env, 1)
    _fire_ar(env, 0)
    if li > 0:
        _emit_vnew_half(env, 1, rope=True, phase=2 * li)
    _emit_chunk(env, 2)
    _emit_chunk(env, 3)
    _fire_ar(env, 1)
    _emit_vnew_half(env, 0, rope=(li < total - 1), phase=2 * li + 1)


def _fire_ar(env, half):
    nc, cfg = env["nc"], env["cfg"]
    src = env["up_hi"] if half else env["up_lo"]
    dst = env["upr_hi"] if half else env["upr_lo"]
    if cfg.get("no_cc"):
        nc.sync.dma_start(dst[:], src[:])
    else:
        nc.gpsimd.collective_compute(
            "AllReduce", ALU.add, replica_groups=env["groups"],
            ins=[src[:].opt()], outs=[dst[:].opt()])


def _emit_vnew_half(env, half, rope=True, phase=0):
    nc, MDT = env["nc"], env["MDT"]
    cfg = env["cfg"]
    D, T = cfg["D"], cfg["T"]
    DT = D // 128
    TT = T // 128
    sm, ps = env["sm"], env["ps"]
    v_sb, vT, ident = env["v_sb"], env["vT"], env["ident"]
    upr = env["upr_hi"] if half else env["upr_lo"]
    wk = env["wk"]
    WDT = env["WDT"]
    t_lo = half * (TT // 2)
    for t in range(t_lo, t_lo + TT // 2):
        r = (t - t_lo) * 128
        # Allocate the staging tile from the ehm stream rotation ("ehst",
        # 4 slots): the WAW buffer-reuse dependency keeps this AllReduce-
        # dependent load ordered AFTER the preceding chunk's DMA stream in
        # the scheduler — otherwise it gets hoisted and head-of-line blocks
        # independent DMAs behind the collective.
        u = wk.tile([128, D], FP, tag="ehst", bufs=4, name="upr")
        nc.sync.dma_start(u[:], upr[r:r + 128])
        _ln_rows(env, v_sb[t], u, D, resid_ap=v_sb[t])
        for i in range(DT):
            ptr = ps.tile([128, 128], MDT, tag="mm", name="ptr2")
            nc.tensor.matmul(ptr[:], v_sb[t][:, i * 128:(i + 1) * 128],
                             ident[:], is_transpose=True, start=True, stop=True)
            nc.vector.tensor_copy(vT[i][:, t * 128:(t + 1) * 128], ptr[:])
    if rope:
        _emit_rope(env, 2 * half)
        _emit_rope(env, 2 * half + 1)


def _emit_chunk(env, g):
    """Attention + MLP for one 512-token chunk; partial update rows written
    to up_lo (chunks 0,1) / up_hi (chunks 2,3)."""
    nc, cfg, MDT = env["nc"], env["cfg"], env["MDT"]
    WDT = env["WDT"]
    D = cfg["D"]
    NH = cfg["N"] // 2
    TC = cfg["TCHUNK"]
    MC = NH // 128
    NSUB = TC // 128
    DT = D // 128
    wk, sm, ps, acc = env["wk"], env["sm"], env["ps"], env["acc"]
    vT, qT, v_sb, latp = env["vT"], env["qT"], env["v_sb"], env["latp"]
    dxh, dyh = env["dxh"], env["dyh"]
    ident, eh_d = env["ident"], env["eh_d"]
    up_d = env["up_hi"] if g >= 2 else env["up_lo"]
    up_r0 = (g - 2) * TC if g >= 2 else g * TC
    XPRE = 3

    t0 = g * TC
    nkb = (g + 1) * NSUB

    def emit_x(m, xrs):
        x_ps = ps.tile([128, TC], FP, tag="mm", name="x_ps")
        for i in range(DT):
            nc.tensor.matmul(x_ps[:],
                             dxh[i][:, m * 128:(m + 1) * 128],
                             vT[i][:, t0:t0 + TC],
                             start=(i == 0), stop=(i == DT - 1))
        xr = wk.tile([128, TC], FP, tag="xr", bufs=4, name="xr")
        nc.scalar.activation(xr[:], x_ps[:], ACTF.Relu)
        xrs[m] = xr

    def emit_av(sb, e_sb, aT_ps):
        for i in range(DT):
            nc.tensor.matmul(aT_ps[:, i, :],
                             v_sb[sb][:, i * 128:(i + 1) * 128],
                             e_sb[:],
                             start=(sb == 0), stop=(sb == nkb - 1),
                             skip_group_check=True)

    # ---- attention: energyT[s, t] blocks, AV lags two blocks
    aT_ps = acc.tile([128, DT, TC], FP, tag="acc", name="aT_ps")
    pend = []
    for sb in range(nkb):
        e_ps = ps.tile([128, TC], FP, tag="mm", name="e_ps")
        for i in range(DT):
            nc.tensor.matmul(e_ps[:],
                             qT[i][:, sb * 128:(sb + 1) * 128],
                             qT[i][:, t0:t0 + TC],
                             start=(i == 0), stop=(i == DT - 1))
        e_sb = wk.tile([128, TC], MDT, tag="esb", bufs=3, name="e_sb")
        nc.vector.tensor_copy(e_sb[:], e_ps[:])
        diag_j = sb - (nkb - NSUB)
        if diag_j >= 0:
            nc.gpsimd.affine_select(e_sb[:], e_sb[:], pattern=[[1, TC]],
                                    compare_op=ALU.is_ge, fill=0.0,
                                    base=-128 * diag_j,
                                    channel_multiplier=-1)
        pend.append((sb, e_sb))
        if len(pend) > 2:
            s0, e0 = pend.pop(0)
            emit_av(s0, e0, aT_ps)
    xrs = {}
    emit_x(0, xrs)
    for s0, e0 in pend:
        emit_av(s0, e0, aT_ps)
    emit_x(1, xrs)
    # ---- aT -> a (PE transposes), LN(a), la -> laT slices ----
    aT_sb = [wk.tile([128, TC], MDT, tag=f"aTs{i}", bufs=2, name="aT_sb")
             for i in range(DT)]
    for i in range(DT):
        nc.scalar.copy(aT_sb[i][:], aT_ps[:, i, :])
    emit_x(2, xrs)
    laT = [latp.tile([128, TC], WDT, tag=f"laTs{i}", name=f"laT{i}")
           for i in range(DT)]
    for tsub in range(NSUB):
        a_ti = ps.tile([128, D], MDT, tag="mm", name="a_ti")
        for i in range(DT):
            nc.tensor.matmul(a_ti[:, i * 128:(i + 1) * 128],
                             aT_sb[i][:, tsub * 128:(tsub + 1) * 128],
                             ident[:], is_transpose=True,
                             start=(i == 0), stop=(i == DT - 1),
                             skip_group_check=True)
        la = sm.tile([128, D], MDT, tag="la", name="la")
        _ln_rows(env, la, a_ti, D)
        for i in range(DT):
            ptr = ps.tile([128, 128], MDT, tag="mm", name="ptr")
            nc.tensor.matmul(ptr[:], la[:, i * 128:(i + 1) * 128], ident[:],
                             is_transpose=True, start=True, stop=True)
            nc.vector.tensor_copy(
                laT[i][:, tsub * 128:(tsub + 1) * 128], ptr[:])
    # ---- MLP: y = relu(Dy^T laT) * x, updateT += Eh^T yel ----
    upT_ps = acc.tile([128, DT, TC], FP, tag="acc", name="upT_ps")

    def emit_up(m, yel, ehm):
        for i in range(DT):
            nc.tensor.matmul(upT_ps[:, i, :],
                             ehm[:, i * 128:(i + 1) * 128],
                             yel[:],
                             start=(m == 0), stop=(m == MC - 1),
                             skip_group_check=True)

    pup = None
    for m in range(MC):
        if env["ehs"] is not None:
            ehm = env["ehs"][m]
        else:
            ehm = wk.tile([128, D], WDT, tag="ehst", bufs=4, name="ehm")
            nc.sync.dma_start(ehm[:], eh_d[m])
        if m + XPRE < MC:
            emit_x(m + XPRE, xrs)
        y_ps = ps.tile([128, TC], FP, tag="mm", name="y_ps")
        for i in range(DT):
            nc.tensor.matmul(y_ps[:],
                             dyh[i][:, m * 128:(m + 1) * 128],
                             laT[i][:],
                             start=(i == 0), stop=(i == DT - 1))
        yr = wk.tile([128, TC], FP, tag="yr", bufs=2, name="yr")
        nc.scalar.activation(yr[:], y_ps[:], ACTF.Relu)
        yel = wk.tile([128, TC], WDT, tag="yel", bufs=2, name="yel")
        nc.vector.tensor_mul(yel[:], yr[:], xrs.pop(m)[:])
        if pup is not None:
            emit_up(*pup)
        pup = (m, yel, ehm)
    emit_up(*pup)
    # ---- upT -> up (PE transposes), evacuate chunk rows ----
    upT_sb = [wk.tile([128, TC], MDT, tag=f"uTs{i}", bufs=1, name="upT_sb")
              for i in range(DT)]
    for i in range(DT):
        nc.scalar.copy(upT_sb[i][:], upT_ps[:, i, :])
    for tsub in range(NSUB):
        u_ti = ps.tile([128, D], MDT, tag="mm", name="u_ti")
        for i in range(DT):
            nc.tensor.matmul(u_ti[:, i * 128:(i + 1) * 128],
                             upT_sb[i][:, tsub * 128:(tsub + 1) * 128],
                             ident[:], is_transpose=True,
                             start=(i == 0), stop=(i == DT - 1),
                             skip_group_check=True)
        u_sb = wk.tile([128, D], FP, tag="usb", bufs=2, name="u_sb")
        nc.vector.tensor_copy(u_sb[:], u_ti[:])
        r0 = up_r0 + tsub * 128
        nc.sync.dma_start(up_d[r0:r0 + 128], u_sb[:])


def _emit_layer(env):
    nc, cfg, MDT = env["nc"], env["cfg"], env["MDT"]
    WDT = env["WDT"]
    T, D = cfg["T"], cfg["D"]
    NH = cfg["N"] // 2
    TC = cfg["TCHUNK"]
    G = T // TC
    TT = T // 128
    MC = NH // 128
    NSUB = TC // 128
    DT = D // 128
    wk, sm, col, ps, acc = env["wk"], env["sm"], env["col"], env["ps"], env["acc"]
    vT, qT, v_sb, latp = env["vT"], env["qT"], env["v_sb"], env["latp"]
    dxh, dyh = env["dxh"], env["dyh"]
    ident, eh_d = env["ident"], env["eh_d"]
    cosT_d, sinT_d = env["cosT_d"], env["sinT_d"]
    up_dram, upr_dram = env["up_dram"], env["upr_dram"]
    XPRE = 3                              # x m-chunks emitted ahead of y

    def emit_x(m, t0, xrs):
        x_ps = ps.tile([128, TC], FP, tag="mm", name="x_ps")
        for i in range(DT):
            nc.tensor.matmul(x_ps[:],
                             dxh[i][:, m * 128:(m + 1) * 128],
                             vT[i][:, t0:t0 + TC],
                             start=(i == 0), stop=(i == DT - 1))
        xr = wk.tile([128, TC], FP, tag="xr", bufs=4, name="xr")
        nc.scalar.activation(xr[:], x_ps[:], ACTF.Relu)
        xrs[m] = xr

    def emit_av(sb, e_sb, aT_ps, nkb):
        # aT[d, t] += v[s, d]^T e[s, t]  (free dim TC keeps f32r at full rate)
        for i in range(DT):
            nc.tensor.matmul(aT_ps[:, i, :],
                             v_sb[sb][:, i * 128:(i + 1) * 128],
                             e_sb[:],
                             start=(sb == 0), stop=(sb == nkb - 1),
                             skip_group_check=True)

    for g in range(G):
        t0 = g * TC
        nkb = (g + 1) * NSUB              # causal key-block count
        # ---- RoPE for this chunk (chunk 0 may have been emitted by prev layer)
        if not (g == 0 and env.pop("_rope0_done", False)):
            _emit_rope(env, g)
        # ---- attention: energyT[s, t] blocks, AV lags two blocks (PE pipelining)
        aT_ps = acc.tile([128, DT, TC], FP, tag="acc", name="aT_ps")
        pend = []
        for sb in range(nkb):
            e_ps = ps.tile([128, TC], FP, tag="mm", name="e_ps")
            for i in range(DT):
                nc.tensor.matmul(e_ps[:],
                                 qT[i][:, sb * 128:(sb + 1) * 128],
                                 qT[i][:, t0:t0 + TC],
                                 start=(i == 0), stop=(i == DT - 1))
            e_sb = wk.tile([128, TC], MDT, tag="esb", bufs=3, name="e_sb")
            nc.vector.tensor_copy(e_sb[:], e_ps[:])
            diag_j = sb - (nkb - NSUB)
            if diag_j >= 0:
                # causal mask: keep where t - s - 128*j >= 0
                nc.gpsimd.affine_select(e_sb[:], e_sb[:], pattern=[[1, TC]],
                                        compare_op=ALU.is_ge, fill=0.0,
                                        base=-128 * diag_j,
                                        channel_multiplier=-1)
            pend.append((sb, e_sb))
            if len(pend) > 2:
                s0, e0 = pend.pop(0)
                emit_av(s0, e0, aT_ps, nkb)
        xrs = {}
        emit_x(0, t0, xrs)
        for s0, e0 in pend:
            emit_av(s0, e0, aT_ps, nkb)
        pend = []
        emit_x(1, t0, xrs)
        # ---- aT -> a (PE transposes), LN(a), la -> laT slices ----
        aT_sb = [wk.tile([128, TC], MDT, tag=f"aTs{i}", bufs=2, name="aT_sb")
                 for i in range(DT)]
        for i in range(DT):
            nc.scalar.copy(aT_sb[i][:], aT_ps[:, i, :])
        emit_x(2, t0, xrs)
        laT = [latp.tile([128, TC], WDT, tag=f"laTs{i}", name=f"laT{i}")
               for i in range(DT)]
        for tsub in range(NSUB):
            a_ti = ps.tile([128, D], MDT, tag="mm", name="a_ti")
            for i in range(DT):
                nc.tensor.matmul(a_ti[:, i * 128:(i + 1) * 128],
                                 aT_sb[i][:, tsub * 128:(tsub + 1) * 128],
                                 ident[:], is_transpose=True,
                                 start=(i == 0), stop=(i == DT - 1),
                                 skip_group_check=True)
            la = sm.tile([128, D], MDT, tag="la", name="la")
            _ln_rows(env, la, a_ti, D)
            for i in range(DT):
                ptr = ps.tile([128, 128], MDT, tag="mm", name="ptr")
                nc.tensor.matmul(ptr[:], la[:, i * 128:(i + 1) * 128], ident[:],
                                 is_transpose=True, start=True, stop=True)
                nc.vector.tensor_copy(
                    laT[i][:, tsub * 128:(tsub + 1) * 128], ptr[:])
        # ---- MLP: y = relu(Dy^T laT) * x, updateT += Eh^T yel, x pipelined ahead
        upT_ps = acc.tile([128, DT, TC], FP, tag="acc", name="upT_ps")

        def emit_up(m, yel, ehm):
            for i in range(DT):
                nc.tensor.matmul(upT_ps[:, i, :],
                                 ehm[:, i * 128:(i + 1) * 128],
                                 yel[:],
                                 start=(m == 0), stop=(m == MC - 1),
                                 skip_group_check=True)

        pup = None
        for m in range(MC):
            if env["ehs"] is not None:
                ehm = env["ehs"][m]
            else:
                ehm = wk.tile([128, D], WDT, tag="ehst", bufs=4, name="ehm")
                nc.sync.dma_start(ehm[:], eh_d[m])
            if m + XPRE < MC:
                emit_x(m + XPRE, t0, xrs)
            y_ps = ps.tile([128, TC], FP, tag="mm", name="y_ps")
            for i in range(DT):
                nc.tensor.matmul(y_ps[:],
                                 dyh[i][:, m * 128:(m + 1) * 128],
                                 laT[i][:],
                                 start=(i == 0), stop=(i == DT - 1))
            yr = wk.tile([128, TC], FP, tag="yr", bufs=2, name="yr")
            nc.scalar.activation(yr[:], y_ps[:], ACTF.Relu)
            yel = wk.tile([128, TC], WDT, tag="yel", bufs=2, name="yel")
            nc.vector.tensor_mul(yel[:], yr[:], xrs.pop(m)[:])
            if pup is not None:
                emit_up(*pup)
            pup = (m, yel, ehm)
        emit_up(*pup)
        # ---- upT -> up (PE transposes), evacuate chunk to DRAM ----
        upT_sb = [wk.tile([128, TC], MDT, tag=f"uTs{i}", bufs=1, name="upT_sb")
                  for i in range(DT)]
        for i in range(DT):
            nc.scalar.copy(upT_sb[i][:], upT_ps[:, i, :])
        for tsub in range(NSUB):
            u_ti = ps.tile([128, D], MDT, tag="mm", name="u_ti")
            for i in range(DT):
                nc.tensor.matmul(u_ti[:, i * 128:(i + 1) * 128],
                                 upT_sb[i][:, tsub * 128:(tsub + 1) * 128],
                                 ident[:], is_transpose=True,
                                 start=(i == 0), stop=(i == DT - 1),
                                 skip_group_check=True)
            u_sb = wk.tile([128, D], FP, tag="usb", bufs=2, name="u_sb")
            nc.vector.tensor_copy(u_sb[:], u_ti[:])
            r0 = t0 + tsub * 128
            nc.sync.dma_start(up_dram[r0:r0 + 128], u_sb[:])
        # ---- AllReduce each completed half so it overlaps later chunks
        if g % 2 == 1 or g == G - 1:
            hs = slice(env.get("_ar_start", 0), (g + 1) * TC)
            env["_ar_start"] = (g + 1) * TC if g < G - 1 else 0
            if cfg.get("no_cc"):
                nc.sync.dma_start(upr_dram[hs], up_dram[hs])
            else:
                nc.gpsimd.collective_compute(
                    "AllReduce", ALU.add, replica_groups=env["groups"],
                    ins=[up_dram[hs].opt()], outs=[upr_dram[hs].opt()])

    # ---- v += LN(update); refresh vT; next layer's chunk-0 rope between halves
    if G > 1:
        nfirst = TC // 128            # rope(0) needs only vT tiles of chunk 0
        _emit_vnew(env, 0, nfirst)
        if env.get("_layers_left", 0) > 0:
            _emit_rope(env, 0)
            env["_rope0_done"] = True
        _emit_vnew(env, nfirst, TT)
    else:
        _emit_vnew(env, 0, TT)


def _emit_vnew(env, t_lo, t_hi):
    nc, MDT = env["nc"], env["MDT"]
    D = env["cfg"]["D"]
    DT = D // 128
    sm, ps = env["sm"], env["ps"]
    v_sb, vT, ident = env["v_sb"], env["vT"], env["ident"]
    upr_dram = env["upr_dram"]
    for t in range(t_lo, t_hi):
        upr = sm.tile([128, D], FP, tag="upr", name="upr")
        nc.sync.dma_start(upr[:], upr_dram[t * 128:(t + 1) * 128])
        _ln_rows(env, v_sb[t], upr, D, resid_ap=v_sb[t])
        for i in range(DT):
            ptr = ps.tile([128, 128], MDT, tag="mm", name="ptr2")
            nc.tensor.matmul(ptr[:], v_sb[t][:, i * 128:(i + 1) * 128], ident[:],
                             is_transpose=True, start=True, stop=True)
            nc.vector.tensor_copy(vT[i][:, t * 128:(t + 1) * 128], ptr[:])


# ====================== host side ======================

_BUILD_CACHE = {}


def shard_inputs(cfg, idx, wte, encoder, decoder_x, decoder_y, readout):
    """Returns the list of per-core input dicts."""
    import ml_dtypes
    wnp = ml_dtypes.bfloat16 if cfg.get("w_dt") == "bf16" else np.float32
    T, D, VOCAB = cfg["T"], cfg["D"], cfg["VOCAB"]
    NH = cfg["N"] // 2
    DT = D // 128
    VT = VOCAB // 128
    heads_per_half = cfg["H"] // 2

    inv_freq = 1.0 / (10000.0 ** (np.arange(0, D, 2, dtype=np.float64) / D))
    tpos = np.arange(T, dtype=np.float64)
    freqs = np.outer(tpos, inv_freq)
    emb = np.concatenate([freqs, freqs], axis=-1)     # [T, D]
    cosT = np.ascontiguousarray(np.cos(emb).T.astype(wnp)).reshape(DT, 128, T)
    sinT = np.ascontiguousarray(np.sin(emb).T.astype(wnp)).reshape(DT, 128, T)
    ident = np.eye(128, dtype=np.float32)

    wte_s = np.ascontiguousarray(wte.astype(np.float32)).reshape(VT, 128, D)
    ro_s = np.ascontiguousarray(readout.astype(wnp)).reshape(DT, 128, VOCAB)

    in_maps = []
    for c in range(cfg["n_cores"]):
        b, h = c // 2, c % 2
        heads = range(h * heads_per_half, (h + 1) * heads_per_half)
        dxh = np.concatenate([decoder_x[hh] for hh in heads], axis=-1)  # [D, NH]
        dyh = np.concatenate([decoder_y[hh] for hh in heads], axis=-1)
        ehh = encoder[h * NH:(h + 1) * NH]                              # [NH, D]
        in_maps.append(dict(
            idxf=np.ascontiguousarray(idx[b].astype(np.float32)[None, :]),
            wte=wte_s,
            dxh=np.ascontiguousarray(dxh.astype(wnp)).reshape(DT, 128, NH),
            dyh=np.ascontiguousarray(dyh.astype(wnp)).reshape(DT, 128, NH),
            eh=np.ascontiguousarray(ehh.astype(wnp)).reshape(NH // 128, 128, D),
            ro=ro_s,
            cosT=cosT, sinT=sinT, ident=ident,
        ))
    return in_maps


def get_program(cfg):
    key = (cfg["T"], cfg["N"], cfg["L"], cfg["mm_dt"], cfg.get("w_dt"),
           cfg["reps"], cfg["n_cores"], cfg["TCHUNK"], cfg.get("no_cc"))
    if key not in _BUILD_CACHE:
        _BUILD_CACHE[key] = build_program(cfg)
    return _BUILD_CACHE[key]


def kernel(idx, wte, encoder, decoder_x, decoder_y, readout):
    cfg = default_cfg()
    nc = get_program(cfg)
    in_maps = shard_inputs(cfg, np.asarray(idx), np.asarray(wte),
                           np.asarray(encoder), np.asarray(decoder_x),
                           np.asarray(decoder_y), np.asarray(readout))
    res = run_bass_kernel_spmd(nc, in_maps, list(range(cfg["n_cores"])))
    B, T, VOCAB = cfg["B"], cfg["T"], cfg["VOCAB"]
    out = np.empty((B, T, VOCAB), np.float32)
    for b in range(B):
        out[b] = res.results[2 * b]["logits"].reshape(T, VOCAB)
    return out

